# revision 1
# baseline (speedup 1.0000x reference)
# Bass/Trainium2 kernel for the masked additive-attention layer
# (nn_AttentionLayer_72258529788543).
#
# Math (per batch b):
#   qp = q @ W1[:, :128].T + b1          [S1, HID]
#   kp = k @ W1[:, 128:].T               [S2, HID]
#   s[i,j] = W2 . relu(qp[i] + kp[j]) + b2
#   A = where(qmask_i & kmask_j, exp(s), 0); attn = A / clip(sum_j A, 2e-15)
#   out = attn @ v
#
# Strategy:
#   * Batch-parallel: 8 batches -> 8 NeuronCores (SPMD, no collectives).
#   * Sparsity: rows with q_mask=0 produce all-zero output; keys with
#     k_mask=0 contribute nothing. Host compacts to the ~50% valid
#     rows/keys, pads to the max count across batches, scatters back.
#   * Device layout: HID on partitions. Per key t, one fused add+relu
#     (DVE tensor_scalar add+max0, or ACT Relu with per-partition bias)
#     produces rt=[128, NQ] bf16; a matmul with a shifted "one-hot W2"
#     stationary accumulates W2 . rt into PSUM row (t mod 128), giving
#     scores_T=[t_block, NQ] directly in the layout the A@V matmul needs.
#   * exp evacuates PSUM->SBUF (ACT, fused +b2 bias). Final matmul
#     A_T.T @ [V | 1] computes both attn@V and the normalizer column in
#     one pass; DVE reciprocal + per-partition scale finishes.
import math
import numpy as np
import ml_dtypes

_B, _S1, _S2, _H = 8, 512, 512, 128

# Keys handled by ACT instead of DVE: none in t-block 0 (lets the ACT
# table load overlap DVE work), then 2 of every 5 keys.
def _use_act(t):
    return t >= 128 and (t % 5) in (1, 3)


_NC_CACHE = {}


def _build(NQ, NK, b2f):
    import concourse.bacc as bacc
    import concourse.tile as tile
    from concourse import mybir
    from contextlib import ExitStack

    f32 = mybir.dt.float32
    bf16 = mybir.dt.bfloat16
    AF = mybir.ActivationFunctionType
    ALU = mybir.AluOpType

    n_tb = (NK + 127) // 128
    n_sb = (NQ + 127) // 128

    nc = bacc.Bacc("TRN2", target_bir_lowering=False, debug=False)
    qcT = nc.dram_tensor("qcT", [128, NQ], f32, kind="ExternalInput").ap()
    kcT = nc.dram_tensor("kcT", [128, NK], f32, kind="ExternalInput").ap()
    vplus = nc.dram_tensor("vplus", [NK, 129], f32, kind="ExternalInput").ap()
    w1qT = nc.dram_tensor("w1qT", [128, 128], f32, kind="ExternalInput").ap()
    w1kT = nc.dram_tensor("w1kT", [128, 128], f32, kind="ExternalInput").ap()
    b1c = nc.dram_tensor("b1c", [128, 1], f32, kind="ExternalInput").ap()
    w2pad = nc.dram_tensor("w2pad", [128, 256], bf16, kind="ExternalInput").ap()
    out = nc.dram_tensor("out", [NQ, 128], f32, kind="ExternalOutput").ap()

    with ExitStack() as ctx:
        tc = ctx.enter_context(tile.TileContext(nc))
        singles = ctx.enter_context(tc.tile_pool(name="singles", bufs=1))
        vpool = ctx.enter_context(tc.tile_pool(name="vpool", bufs=n_tb))
        atpool = ctx.enter_context(tc.tile_pool(name="atpool", bufs=n_tb))
        rtpool = ctx.enter_context(tc.tile_pool(name="rtpool", bufs=8))
        opool = ctx.enter_context(tc.tile_pool(name="opool", bufs=2))
        pp1 = ctx.enter_context(tc.tile_pool(name="pp1", bufs=1, space="PSUM"))
        pps = ctx.enter_context(tc.tile_pool(name="pps", bufs=2, space="PSUM"))
        ppo = ctx.enter_context(tc.tile_pool(name="ppo", bufs=2, space="PSUM"))

        sb_w1qT = singles.tile([128, 128], f32)
        nc.sync.dma_start(out=sb_w1qT, in_=w1qT)
        sb_qcT = singles.tile([128, NQ], f32)
        nc.sync.dma_start(out=sb_qcT, in_=qcT)
        sb_w1kT = singles.tile([128, 128], f32)
        nc.sync.dma_start(out=sb_w1kT, in_=w1kT)
        sb_kcT = singles.tile([128, NK], f32)
        nc.sync.dma_start(out=sb_kcT, in_=kcT)
        sb_b1 = singles.tile([128, 1], f32)
        nc.sync.dma_start(out=sb_b1, in_=b1c)
        sb_w2pad = singles.tile([128, 256], bf16)
        nc.sync.dma_start(out=sb_w2pad, in_=w2pad)
        sb_vp = []
        for tb in range(n_tb):
            bs = min(128, NK - tb * 128)
            v = vpool.tile([128, 129], f32)
            nc.sync.dma_start(out=v[:bs], in_=vplus[tb * 128 : tb * 128 + bs, :])
            sb_vp.append(v)

        # Phase 1: projections. qp_T = W1q @ qc_T + b1, kp_T = W1k @ kc_T.
        ps_q = pp1.tile([128, NQ], f32)
        nc.tensor.matmul(ps_q, lhsT=sb_w1qT, rhs=sb_qcT, start=True, stop=True)
        sb_qpT = singles.tile([128, NQ], bf16)
        nc.scalar.activation(
            out=sb_qpT, in_=ps_q, func=AF.Identity, bias=sb_b1[:, 0:1], scale=1.0
        )
        ps_k = pp1.tile([128, NK], f32)
        nc.tensor.matmul(ps_k, lhsT=sb_w1kT, rhs=sb_kcT, start=True, stop=True)
        # fp32: used as DVE tensor_scalar scalar1 / ACT bias (both need f32)
        sb_kpT = singles.tile([128, NK], f32)
        nc.scalar.copy(out=sb_kpT, in_=ps_k)

        # Phase 2: per key t, rt = relu(qp_T + kp_T[:, t]) (bf16), then
        # scores_T[t mod 128, :] += W2 . rt via shifted one-hot stationary.
        aT = []
        for tb in range(n_tb):
            bs = min(128, NK - tb * 128)
            ps_s = pps.tile([128, NQ], f32)
            for j in range(bs):
                t = tb * 128 + j
                rt = rtpool.tile([128, NQ], bf16)
                if _use_act(t):
                    nc.scalar.activation(
                        out=rt, in_=sb_qpT, func=AF.Relu, bias=sb_kpT[:, t : t + 1]
                    )
                else:
                    nc.vector.tensor_scalar(
                        out=rt,
                        in0=sb_qpT,
                        scalar1=sb_kpT[:, t : t + 1],
                        scalar2=0.0,
                        op0=ALU.add,
                        op1=ALU.max,
                    )
                nc.tensor.matmul(
                    out=ps_s,
                    lhsT=sb_w2pad[:, 128 - j : 256 - j],
                    rhs=rt,
                    start=(j == 0),
                    stop=(j == bs - 1),
                )
            a = atpool.tile([128, NQ], f32)
            nc.scalar.activation(out=a[:bs], in_=ps_s[:bs], func=AF.Exp, bias=b2f)
            aT.append((a, bs))

        # Phase 3: psum_o[:, 0:128] = A.T.T @ V = attn-unnormalized @ V,
        # psum_o[:, 128] = row sums; normalize and store.
        for si in range(n_sb):
            qs = min(128, NQ - si * 128)
            ps_o = ppo.tile([128, 129], f32)
            for tb, (a, bs) in enumerate(aT):
                nc.tensor.matmul(
                    out=ps_o[:qs],
                    lhsT=a[:bs, si * 128 : si * 128 + qs],
                    rhs=sb_vp[tb][:bs],
                    start=(tb == 0),
                    stop=(tb == n_tb - 1),
                )
            rec = opool.tile([128, 1], f32)
            nc.vector.tensor_scalar_max(rec[:qs], ps_o[:qs, 128:129], 2e-15)
            nc.vector.reciprocal(rec[:qs], rec[:qs])
            ob = opool.tile([128, 128], f32)
            nc.vector.tensor_scalar_mul(ob[:qs], ps_o[:qs, 0:128], rec[:qs, 0:1])
            nc.sync.dma_start(out=out[si * 128 : si * 128 + qs, :], in_=ob[:qs])

    nc.compile()
    return nc


def _prepare(query, key, value, q_mask, k_mask, W1, b1, W2, b2):
    """Compact per-batch valid rows/keys; build per-core input maps."""
    bf = ml_dtypes.bfloat16
    idx_q = [np.nonzero(q_mask[b])[0] for b in range(_B)]
    idx_k = [np.nonzero(k_mask[b])[0] for b in range(_B)]
    nq_max = max(len(i) for i in idx_q)
    nk_max = max(len(i) for i in idx_k)
    if nq_max == 0 or nk_max == 0:
        return None, idx_q, 0, 0
    NQ = max(8, ((nq_max + 7) // 8) * 8)
    NK = max(8, ((nk_max + 7) // 8) * 8)

    w1qT = np.ascontiguousarray(W1[:, :_H].T, dtype=np.float32)
    w1kT = np.ascontiguousarray(W1[:, _H:].T, dtype=np.float32)
    b1c = np.ascontiguousarray(b1.reshape(_H, 1), dtype=np.float32)
    w2pad = np.zeros((_H, 256), dtype=bf)
    w2pad[:, 128] = W2[0].astype(bf)

    in_maps = []
    for b in range(_B):
        iq, ik = idx_q[b], idx_k[b]
        qcT = np.zeros((_H, NQ), np.float32)
        qcT[:, : len(iq)] = query[b, iq].T
        kcT = np.zeros((_H, NK), np.float32)
        kcT[:, : len(ik)] = key[b, ik].T
        vplus = np.zeros((NK, 129), np.float32)
        vplus[: len(ik), :_H] = value[b, ik]
        vplus[: len(ik), _H] = 1.0
        in_maps.append(
            dict(
                qcT=qcT,
                kcT=kcT,
                vplus=vplus,
                w1qT=w1qT,
                w1kT=w1kT,
                b1c=b1c,
                w2pad=w2pad,
            )
        )
    return in_maps, idx_q, NQ, NK


def run(inputs, trace=False):
    """Returns (full_output, BassKernelResults | None)."""
    from concourse import bass_utils

    query = np.asarray(inputs["query"], np.float32)
    key = np.asarray(inputs["key"], np.float32)
    value = np.asarray(inputs["value"], np.float32)
    q_mask = np.asarray(inputs["q_mask"])
    k_mask = np.asarray(inputs["k_mask"])
    W1 = np.asarray(inputs["W1"], np.float32)
    b1 = np.asarray(inputs["b1"], np.float32)
    W2 = np.asarray(inputs["W2"], np.float32)
    b2 = np.asarray(inputs["b2"], np.float32)

    out = np.zeros((_B, _S1, _H), np.float32)
    in_maps, idx_q, NQ, NK = _prepare(
        query, key, value, q_mask, k_mask, W1, b1, W2, b2
    )
    if in_maps is None:
        return out, None

    cache_key = (NQ, NK, float(b2[0]))
    nc = _NC_CACHE.get(cache_key)
    if nc is None:
        nc = _build(NQ, NK, float(b2[0]))
        _NC_CACHE[cache_key] = nc

    res = bass_utils.run_bass_kernel_spmd(
        nc, in_maps, core_ids=list(range(_B)), trace=trace
    )
    for b in range(_B):
        iq = idx_q[b]
        if len(iq):
            out[b, iq, :] = res.results[b]["out"][: len(iq)]
    return out, res


def kernel(**inputs):
    out, _ = run(inputs)
    return out

